# revision 26
# baseline (speedup 1.0000x reference)
"""Distributed Trainium2 kernel for cross-attention (nn_Attention_50732153701013).

Reference computation (b=2, n=2048, dim=1024, heads=16, d_head=64):
    qkv  = split(x  @ W_qkv)          -> q,  k,  v
    qkv1 = split(x1 @ W_qkv)          -> q1, k1, v1
    out  = merge(softmax(q  k1^T / 8) v1) @ W_out + b_out
    out1 = merge(softmax(q1 k ^T / 8) v ) @ W_out + b_out

Sharding over 8 cores: core c handles batch (c // 4) and heads
[(c%4)*4, (c%4)*4+4).  Each core computes its 4 heads' attention for both
cross directions plus the partial out-projection (row-slice of W_out);
the host sums the 4 partial outputs per batch.

Device-side design notes:
  * x/x1 are pre-transposed AND pre-cast to bf16 on the host (xT [dim, n]),
    so every matmul contraction axis is on SBUF partitions and input DMA
    traffic is halved.  W_qkv/W_out are bf16; partial outputs leave as bf16
    and are summed in f32 on the host.
  * Scores are computed transposed: S^T[m, n] = k[m]*q[n], so the softmax
    reduction axis (m) lies on PSUM partitions.  exp() needs no max
    subtraction (|scores| <~ 6).  The softmax denominator is obtained by
    appending a ones-column to V, so the AV matmul also yields
    colsum(exp S^T) as PSUM row 64.
  * Head PAIRING on the PE array: head 2*hp lives on SBUF partitions 0-63
    and head 2*hp+1 on partitions 64-127.  Their K=64 score matmuls carry
    tile_position (0,0)/(64,0) (auto-derived from base partitions) and are
    emitted back-to-back into ONE [128,1024] psum tile (different banks),
    so they execute CONCURRENTLY on the two row-halves of the 128x128
    array (64x128 row tiling) and one ACT evacuates both heads.
  * The kernel is exp-(Scalar-engine-)bound: 256 ACTIVATEs of [128,1024]
    ~= 285us busy.  All projection work (QKV chains, V tiles, output
    projections) is therefore expressed as GENERATORS yielding ~0.5-1.7us
    pieces, and the attention unit PUMPS a bounded budget of filler
    between m-chunk iterations so the PE's slack inside the ACT-bound
    phase absorbs the projections without ever delaying the next score
    pair.  Tile deps only point backward in emission order, so explicit
    drain points force-finish generators right before the first unit that
    reads their output.
  * PSUM budget (8 banks): scores 2x[128,1024] (4), AV accumulators
    2x[128,512] (2), shared transient ring 2x[128,512] (2) used by QKV
    chains + V tiles + out-projěction pieces + the denominator broadcast.
"""

import numpy as np
import ml_dtypes

B, N, DIM = 2, 2048, 1024
HEADS, DHEAD = 16, 64
H_LOC = 4                 # heads per core
INNER_LOC = H_LOC * DHEAD  # 256
NCORES = 8
SCALE = DHEAD ** -0.5     # 0.125

BF = ml_dtypes.bfloat16

_CACHED = {}


def _build_graph():
    import concourse.mybir as mybir
    from concourse import bacc
    from concourse.tile import TileContext

    f32 = mybir.dt.float32
    bf16 = mybir.dt.bfloat16
    AF = mybir.ActivationFunctionType

    nc = bacc.Bacc(None, target_bir_lowering=False)

    xT = nc.dram_tensor("xT", [DIM, N], bf16, kind="ExternalInput")
    x1T = nc.dram_tensor("x1T", [DIM, N], bf16, kind="ExternalInput")
    wqkv = nc.dram_tensor("wqkv", [DIM, 3 * INNER_LOC], bf16,
                          kind="ExternalInput")
    wout = nc.dram_tensor("wout", [INNER_LOC, DIM], bf16, kind="ExternalInput")
    out = nc.dram_tensor("out", [2, N, DIM], bf16, kind="ExternalOutput")

    KO = DIM // 128            # 8 contraction chunks for the projections
    NT = N // 128              # 16 n tiles / m chunks
    VW = DHEAD + 1             # 65: head slice width in v_sb (data + ones col)

    with TileContext(nc) as tc:
        with (
            nc.allow_low_precision(reason="bf16 matmul operands, fp32 accum"),
            tc.tile_pool(name="persist", bufs=1) as persist,
            tc.tile_pool(name="qk", bufs=1) as qkpool,
            tc.tile_pool(name="xstage", bufs=4) as xstage,
            tc.tile_pool(name="attn", bufs=6) as attn,
            tc.tile_pool(name="otp", bufs=6) as otp,
            tc.tile_pool(name="outstage", bufs=4) as outstage,
            tc.tile_pool(name="ps_s", bufs=2, space="PSUM") as ps_s,
            tc.tile_pool(name="ps_o", bufs=2, space="PSUM") as ps_o,
            tc.tile_pool(name="ps_out", bufs=2, space="PSUM") as ps_out,
        ):
            wqkv_sb = persist.tile([128, KO, 3 * INNER_LOC], bf16)
            nc.sync.dma_start(
                wqkv_sb[:], wqkv.rearrange("(ko p) c -> p ko c", p=128)
            )
            wout_sb = persist.tile([128, 2, DIM], bf16)
            ones_f32 = persist.tile([128, 1], f32)
            nc.any.memset(ones_f32[:], 1.0)
            ones_row = persist.tile([1, 64], bf16)
            nc.vector.tensor_copy(
                ones_row[:], ones_f32[0:1, :].broadcast_to([1, 64])
            )

            # transposed q/k for both inputs: [128, chunk(2), n]
            qT = qkpool.tile([128, 2, N], bf16, tag="qT")
            kT = qkpool.tile([128, 2, N], bf16, tag="kT")
            q1T = qkpool.tile([128, 2, N], bf16, tag="q1T")
            k1T = qkpool.tile([128, 2, N], bf16, tag="k1T")
            # v in [m, head-slices] layout, ones col per head at offset 64
            v_sb = persist.tile([128, NT, H_LOC * VW], bf16, tag="v")
            v1_sb = persist.tile([128, NT, H_LOC * VW], bf16, tag="v1")
            for vt in (v_sb, v1_sb):
                nc.vector.tensor_copy(
                    vt[:].rearrange("p t (h c) -> p t h c", h=H_LOC)[:, :, :, DHEAD:],
                    ones_f32[:, None, None, :].broadcast_to([128, NT, H_LOC, 1]),
                )

            # ---------------- filler pump machinery ----------------
            # Generators yield their approximate PE cost in ns after each
            # emitted piece.  attention() pumps a budget's worth between
            # m-chunk pairs; drain() force-finishes gens whose outputs an
            # upcoming emission reads.
            from collections import deque
            fq = deque()

            def pump(budget):
                while budget > 0 and fq:
                    try:
                        budget -= next(fq[0])
                    except StopIteration:
                        fq.popleft()

            def drain(*gens):
                for g in gens:
                    for _ in g:
                        pass
                    if g in fq:
                        fq.remove(g)

            # ---------------- QKV projection generators ----------------
            def load_xs(srcT, half, q):
                nslc = slice(half * 1024, (half + 1) * 1024)
                xs = xstage.tile([128, KO, 1024], bf16, tag="xs")
                src_r = srcT.rearrange("(ko p) n -> p ko n", p=128)
                # split the load so early matmuls start sooner
                q.dma_start(xs[:, 0:2, :], src_r[:, 0:2, nslc])
                q.dma_start(xs[:, 2:KO, :], src_r[:, 2:KO, nslc])
                return xs

            def qk_gen(xs, qdst, kdst, half, part):
                """One q/k chunk projection as [128,512] chains on the
                transient psum ring.  part: q|q0|q1|k|k0|k1."""
                mbs = {"q": [0, 1], "q0": [0], "q1": [1],
                       "k": [2, 3], "k0": [2], "k1": [3]}[part]
                for mb in mbs:
                    dst = qdst if mb < 2 else kdst
                    ci = mb % 2
                    for nb in range(2):
                        ps = ps_out.tile([128, 512], f32, tag="ps_out")
                        for ko in range(KO):
                            nc.tensor.matmul(
                                ps[:],
                                wqkv_sb[:, ko, mb * 128:(mb + 1) * 128],
                                xs[:, ko, nb * 512:(nb + 1) * 512],
                                start=(ko == 0),
                                stop=(ko == KO - 1),
                            )
                        off = half * 1024 + nb * 512
                        nc.vector.tensor_copy(
                            dst[:, ci, off:off + 512], ps[:])
                        yield 1750

            def v_gen(xs, vdst, half):
                for nt in range(8):
                    nt_g = half * 8 + nt
                    ps = ps_out.tile([128, 512], f32, tag="ps_out")
                    for ko in range(KO):
                        nc.tensor.matmul(
                            ps[:, 0:INNER_LOC],
                            xs[:, ko, nt * 128:(nt + 1) * 128],
                            wqkv_sb[:, ko, 2 * INNER_LOC:3 * INNER_LOC],
                            start=(ko == 0),
                            stop=(ko == KO - 1),
                        )
                    nc.vector.tensor_copy(
                        vdst[:, nt_g, :]
                        .rearrange("p (h c) -> p h c", h=H_LOC)[:, :, :DHEAD],
                        ps[:, 0:INNER_LOC].rearrange("p (h c) -> p h c",
                                                     h=H_LOC),
                    )
                    yield 950

            # ---------------- attention unit + out-projection ----------------
            ots = {}

            def attention(nb, d, hps=(0, 1), budget=900, drain_at=None):
                """One n-block of one cross direction for one head pair."""
                nslc = slice(nb * 512, (nb + 1) * 512)
                qsrc, ksrc, vsrc = (
                    (qT, k1T, v1_sb) if d == 0 else (q1T, kT, v_sb)
                )
                if (nb, d) in ots:
                    ot = ots[(nb, d)]
                else:
                    ot = otp.tile([128, 2, 512], bf16, tag="ot")
                    ots[(nb, d)] = ot
                for hp in hps:
                    hE, hO = 2 * hp, 2 * hp + 1
                    poE = ps_o.tile([128, 512], f32, tag="po")
                    poO = ps_o.tile([128, 512], f32, tag="po")
                    for mcp in range(8):
                        if drain_at and mcp in drain_at:
                            drain(*drain_at.pop(mcp))
                        for j in range(2):
                            mc = mcp * 2 + j
                            mslc = slice(mc * 128, (mc + 1) * 128)
                            ps = ps_s.tile([128, 1024], f32, tag="ps_s")
                            nc.tensor.matmul(
                                ps[:, 0:512],
                                ksrc[0:64, hp, mslc],
                                qsrc[0:64, hp, nslc],
                                start=True, stop=True,
                            )
                            nc.tensor.matmul(
                                ps[:, 512:1024],
                                ksrc[64:128, hp, mslc],
                                qsrc[64:128, hp, nslc],
                                start=True, stop=True,
                            )
                            a = attn.tile([128, 1024], bf16, tag="a")
                            nc.scalar.activation(a[:], ps[:], AF.Exp,
                                                 scale=SCALE)
                            nc.tensor.matmul(
                                poE[0:VW, :],
                                vsrc[:, mc, hE * VW:(hE + 1) * VW],
                                a[:, 0:512],
                                start=(mc == 0), stop=(mc == NT - 1),
                            )
                            nc.tensor.matmul(
                                poO[0:VW, :],
                                vsrc[:, mc, hO * VW:(hO + 1) * VW],
                                a[:, 512:1024],
                                start=(mc == 0), stop=(mc == NT - 1),
                            )
                        pump(budget)
                    for po, prow in ((poE, slice(0, 64)),
                                     (poO, slice(64, 128))):
                        csrow = attn.tile([1, 512], f32, tag="csrow")
                        nc.vector.tensor_copy(csrow[:], po[64:65, :])
                        recip_f = attn.tile([1, 512], f32, tag="recip_f")
                        nc.vector.reciprocal_approx_fast(
                            out=recip_f[:], in_=csrow[:]
                        )
                        recip = attn.tile([1, 512], bf16, tag="recip")
                        nc.vector.tensor_copy(recip[:], recip_f[:])
                        pbt = ps_out.tile([128, 512], f32, tag="ps_out")
                        nc.tensor.matmul(pbt[0:64, :], ones_row[:], recip[:],
                                         start=True, stop=True)
                        nc.vector.tensor_copy(ot[prow, hp, :], po[0:64, :])
                        nc.vector.tensor_mul(
                            ot[prow, hp, :], ot[prow, hp, :], pbt[0:64, :]
                        )

            def proj_gen(nb, d):
                """Out-projection of one (nb, d) in per-(nt, db) pieces."""
                ot = ots[(nb, d)]
                for nt in range(4):
                    ob = outstage.tile([128, DIM], f32, tag="ob")
                    for db in range(2):
                        ps = ps_out.tile([128, 512], f32, tag="ps_out")
                        for ki in range(2):
                            nc.tensor.matmul(
                                ps[:],
                                ot[:, ki, nt * 128:(nt + 1) * 128],
                                wout_sb[:, ki, db * 512:(db + 1) * 512],
                                start=(ki == 0), stop=(ki == 1))
                        nc.vector.tensor_copy(
                            ob[:, db * 512:(db + 1) * 512], ps[:]
                        )
                        yield 550
                    obh = outstage.tile([128, DIM], bf16, tag="obh")
                    nc.vector.tensor_copy(obh[:], ob[:])
                    nc.sync.dma_start(
                        out[d, nb * 512 + nt * 128:
                            nb * 512 + (nt + 1) * 128, :],
                        obh[:],
                    )
                del ots[(nb, d)]

            def projki_gen(nb, d, ki):
                """One contraction half of (nb, d)'s projection.  ki=0
                stages f32 partials; ki=1 adds and ships out."""
                for nt in range(4):
                    if ki == 0:
                        ob = outstage.tile([128, DIM], f32, tag="ob")
                        ots[(nb, d, "ob", nt)] = ob
                    else:
                        ob = ots.pop((nb, d, "ob", nt))
                    for db in range(2):
                        ps = ps_out.tile([128, 512], f32, tag="ps_out")
                        nc.tensor.matmul(
                            ps[:],
                            ots[(nb, d)][:, ki, nt * 128:(nt + 1) * 128],
                            wout_sb[:, ki, db * 512:(db + 1) * 512],
                            start=True, stop=True)
                        if ki == 0:
                            nc.vector.tensor_copy(
                                ob[:, db * 512:(db + 1) * 512], ps[:])
                        else:
                            nc.vector.tensor_add(
                                ob[:, db * 512:(db + 1) * 512],
                                ob[:, db * 512:(db + 1) * 512], ps[:])
                        yield 300
                    if ki == 1:
                        obh = outstage.tile([128, DIM], bf16, tag="obh")
                        nc.vector.tensor_copy(obh[:], ob[:])
                        nc.sync.dma_start(
                            out[d, nb * 512 + nt * 128:
                                nb * 512 + (nt + 1) * 128, :],
                            obh[:],
                        )

            # ---------------- emission ----------------
            # Tile deps only point backward in emission order, so this is a
            # valid sequential program: every generator is force-drained
            # (or arithmetically guaranteed pumped) before the first unit
            # that reads its output.  Units run hp-major so each unit
            # introduces only a small set of new dependencies.
            xs00 = load_xs(xT, 0, nc.gpsimd)
            xs01 = load_xs(xT, 1, nc.scalar)
            xs10 = load_xs(x1T, 0, nc.sync)
            xs11 = load_xs(x1T, 1, nc.sync)
            nc.sync.dma_start(
                wout_sb[:], wout.rearrange("(ki p) d -> p ki d", p=128)
            )
            # prefix: minimum work before the first exp can fire
            drain(qk_gen(xs00, qT, kT, 0, "k0"))
            drain(v_gen(xs00, v_sb, 0))
            drain(qk_gen(xs10, q1T, k1T, 0, "q0"))
            # U1: first unit carries its own half-1 deps in the pump queue
            g_k0h1 = qk_gen(xs01, qT, kT, 1, "k0")
            g_vh1 = v_gen(xs01, v_sb, 1)
            fq.extend([g_k0h1, g_vh1])
            attention(0, 1, hps=(0,), budget=1800,
                      drain_at={4: [g_k0h1, g_vh1]})
            g_q1h1 = qk_gen(xs11, q1T, k1T, 1, "q0")
            fq.append(g_q1h1)
            attention(1, 1, hps=(0,))
            g_k1h0 = qk_gen(xs00, qT, kT, 0, "k1")
            g_k1h1 = qk_gen(xs01, qT, kT, 1, "k1")
            g_q1b0 = qk_gen(xs10, q1T, k1T, 0, "q1")
            fq.extend([g_k1h0, g_k1h1, g_q1b0])
            drain(g_q1h1)
            attention(2, 1, hps=(0,))
            g_q1b1 = qk_gen(xs11, q1T, k1T, 1, "q1")
            fq.append(g_q1b1)
            attention(3, 1, hps=(0,))
            drain(g_k1h0, g_k1h1, g_q1b0)
            attention(0, 1, hps=(1,))
            g_xq0 = qk_gen(xs00, qT, kT, 0, "q")
            fq.append(g_xq0)
            drain(g_q1b1)
            attention(1, 1, hps=(1,))
            g_p01 = proj_gen(0, 1)
            g_k1_0 = qk_gen(xs10, q1T, k1T, 0, "k")
            fq.extend([g_p01, g_k1_0])
            attention(2, 1, hps=(1,))
            g_p11 = proj_gen(1, 1)
            g_v1h0 = v_gen(xs10, v1_sb, 0)
            g_k10h1 = qk_gen(xs11, q1T, k1T, 1, "k0")
            g_v1h1 = v_gen(xs11, v1_sb, 1)
            fq.extend([g_p11, g_v1h0, g_k10h1, g_v1h1])
            attention(3, 1, hps=(1,))
            drain(g_xq0, g_k1_0, g_v1h0, g_v1h1)
            attention(0, 0, hps=(0,), drain_at={4: [g_k10h1]})
            g_xq1 = qk_gen(xs01, qT, kT, 1, "q")
            g_k11h1 = qk_gen(xs11, q1T, k1T, 1, "k1")
            fq.extend([g_xq1, g_k11h1])
            attention(1, 0, hps=(0,))
            g_p21 = proj_gen(2, 1)
            fq.append(g_p21)
            # ot(2,0)/(3,0) recycle the otp ring slots of ot(0,1)/(1,1):
            # those projections must be fully emitted first
            drain(g_xq1, g_p01)
            attention(2, 0, hps=(0,))
            g_p31 = proj_gen(3, 1)
            fq.append(g_p31)
            drain(g_p11)
            attention(3, 0, hps=(0,))
            drain(g_k11h1)
            attention(0, 0, hps=(1,))
            g_p00 = proj_gen(0, 0)
            fq.append(g_p00)
            attention(1, 0, hps=(1,))
            g_p10 = proj_gen(1, 0)
            fq.append(g_p10)
            attention(2, 0, hps=(1,))
            g_p20 = proj_gen(2, 0)
            g_pk0 = projki_gen(3, 0, 0)
            fq.extend([g_p20, g_pk0])
            attention(3, 0, hps=(1,))
            drain(*list(fq))
            drain(projki_gen(3, 0, 1))
            del ots[(3, 0)]
    return nc


def _get_graph():
    if "nc" not in _CACHED:
        nc = _build_graph()
        # Bacc defers register allocation to finalize(); the pjrt exec path
        # serializes nc.m directly, so finalize here.
        nc.finalize()
        _CACHED["nc"] = nc
    return _CACHED["nc"]


def _make_in_maps(x, x1, W_qkv, W_out):
    in_maps = []
    xTb = [np.ascontiguousarray(x[b].T).astype(BF) for b in range(B)]
    x1Tb = [np.ascontiguousarray(x1[b].T).astype(BF) for b in range(B)]
    for c in range(NCORES):
        b = c // 4
        h0 = (c % 4) * H_LOC
        cols = np.concatenate(
            [W_qkv[:, j * DIM + h0 * DHEAD: j * DIM + (h0 + H_LOC) * DHEAD]
             for j in range(3)],
            axis=1,
        )
        in_maps.append({
            "xT": xTb[b],
            "x1T": x1Tb[b],
            "wqkv": np.ascontiguousarray(cols).astype(BF),
            "wout": np.ascontiguousarray(
                W_out[h0 * DHEAD:(h0 + H_LOC) * DHEAD, :]
            ).astype(BF),
        })
    return in_maps


def _run(x, x1, W_qkv, W_out, b_out, **spmd_kwargs):
    from concourse.bass_utils import run_bass_kernel_spmd

    nc = _get_graph()
    in_maps = _make_in_maps(x, x1, W_qkv, W_out)
    res = run_bass_kernel_spmd(nc, in_maps, core_ids=list(range(NCORES)),
                               **spmd_kwargs)
    parts = [r["out"].view(BF).reshape(2, N, DIM) for r in res.results]
    out = np.zeros((B, N, DIM), np.float32)
    out1 = np.zeros((B, N, DIM), np.float32)
    for b in range(B):
        grp = parts[4 * b:4 * b + 4]
        out[b] = sum(np.float32(p[0]) for p in grp) + b_out
        out1[b] = sum(np.float32(p[1]) for p in grp) + b_out
    return (out, out1), res


def kernel(x, x1, W_qkv, W_out, b_out):
    x = np.asarray(x, np.float32)
    x1 = np.asarray(x1, np.float32)
    W_qkv = np.asarray(W_qkv, np.float32)
    W_out = np.asarray(W_out, np.float32)
    b_out = np.asarray(b_out, np.float32)
    (out, out1), _ = _run(x, x1, W_qkv, W_out, b_out)
    return out, out1


# revision 32
# speedup vs baseline: 1.0081x; 1.0081x over previous
"""Distributed Trainium2 kernel for cross-attention (nn_Attention_50732153701013).

Reference computation (b=2, n=2048, dim=1024, heads=16, d_head=64):
    qkv  = split(x  @ W_qkv)          -> q,  k,  v
    qkv1 = split(x1 @ W_qkv)          -> q1, k1, v1
    out  = merge(softmax(q  k1^T / 8) v1) @ W_out + b_out
    out1 = merge(softmax(q1 k ^T / 8) v ) @ W_out + b_out

Sharding over 8 cores: core c handles batch (c // 4) and heads
[(c%4)*4, (c%4)*4+4).  Each core computes its 4 heads' attention for both
cross directions plus the partial out-projection (row-slice of W_out);
the host sums the 4 partial outputs per batch.

Device-side design notes:
  * x/x1 are pre-transposed AND pre-cast to bf16 on the host (xT [dim, n]),
    so every matmul contraction axis is on SBUF partitions and input DMA
    traffic is halved.  W_qkv/W_out are bf16; partial outputs leave as bf16
    and are summed in f32 on the host.
  * Scores are computed transposed: S^T[m, n] = k[m]*q[n], so the softmax
    reduction axis (m) lies on PSUM partitions.  exp() needs no max
    subtraction (|scores| <~ 6).  The softmax denominator is obtained by
    appending a ones-column to V, so the AV matmul also yields
    colsum(exp S^T) as PSUM row 64.
  * Head PAIRING on the PE array: head 2*hp lives on SBUF partitions 0-63
    and head 2*hp+1 on partitions 64-127.  Their K=64 score matmuls carry
    tile_position (0,0)/(64,0) (auto-derived from base partitions) and are
    emitted back-to-back into ONE [128,1024] psum tile (different banks),
    so they execute CONCURRENTLY on the two row-halves of the 128x128
    array (64x128 row tiling) and one ACT evacuates both heads.
  * The kernel is exp-(Scalar-engine-)bound: 256 ACTIVATEs of [128,1024]
    ~= 285us busy.  All projection work (QKV chains, V tiles, output
    projections) is therefore expressed as GENERATORS yielding ~0.5-1.7us
    pieces, and the attention unit PUMPS a bounded budget of filler
    between m-chunk iterations so the PE's slack inside the ACT-bound
    phase absorbs the projections without ever delaying the next score
    pair.  Tile deps only point backward in emission order, so explicit
    drain points force-finish generators right before the first unit that
    reads their output.
  * PSUM budget (8 banks): scores 2x[128,1024] (4), AV accumulators
    2x[128,512] (2), shared transient ring 2x[128,512] (2) used by QKV
    chains + V tiles + out-projěction pieces + the denominator broadcast.
"""

import numpy as np
import ml_dtypes

B, N, DIM = 2, 2048, 1024
HEADS, DHEAD = 16, 64
H_LOC = 4                 # heads per core
INNER_LOC = H_LOC * DHEAD  # 256
NCORES = 8
SCALE = DHEAD ** -0.5     # 0.125

BF = ml_dtypes.bfloat16

_CACHED = {}


def _build_graph():
    import concourse.mybir as mybir
    from concourse import bacc
    from concourse.tile import TileContext

    f32 = mybir.dt.float32
    bf16 = mybir.dt.bfloat16
    AF = mybir.ActivationFunctionType

    nc = bacc.Bacc(None, target_bir_lowering=False)

    xT = nc.dram_tensor("xT", [DIM, N], bf16, kind="ExternalInput")
    x1T = nc.dram_tensor("x1T", [DIM, N], bf16, kind="ExternalInput")
    wqkv = nc.dram_tensor("wqkv", [DIM, 3 * INNER_LOC], bf16,
                          kind="ExternalInput")
    wout = nc.dram_tensor("wout", [INNER_LOC, DIM], bf16, kind="ExternalInput")
    out = nc.dram_tensor("out", [2, N, DIM], bf16, kind="ExternalOutput")

    KO = DIM // 128            # 8 contraction chunks for the projections
    NT = N // 128              # 16 n tiles / m chunks
    VW = DHEAD + 1             # 65: head slice width in v_sb (data + ones col)

    with TileContext(nc) as tc:
        with (
            nc.allow_low_precision(reason="bf16 matmul operands, fp32 accum"),
            tc.tile_pool(name="persist", bufs=1) as persist,
            tc.tile_pool(name="qk", bufs=1) as qkpool,
            tc.tile_pool(name="xstage", bufs=4) as xstage,
            tc.tile_pool(name="attn", bufs=6) as attn,
            tc.tile_pool(name="otp", bufs=6) as otp,
            tc.tile_pool(name="outstage", bufs=4) as outstage,
            tc.tile_pool(name="ps_s", bufs=2, space="PSUM") as ps_s,
            tc.tile_pool(name="ps_o", bufs=2, space="PSUM") as ps_o,
            tc.tile_pool(name="ps_out", bufs=2, space="PSUM") as ps_out,
        ):
            wqkv_sb = persist.tile([128, KO, 3 * INNER_LOC], bf16)
            nc.sync.dma_start(
                wqkv_sb[:], wqkv.rearrange("(ko p) c -> p ko c", p=128)
            )
            wout_sb = persist.tile([128, 2, DIM], bf16)
            ones_f32 = persist.tile([128, 1], f32)
            nc.any.memset(ones_f32[:], 1.0)
            ones_row = persist.tile([1, 64], bf16)
            nc.vector.tensor_copy(
                ones_row[:], ones_f32[0:1, :].broadcast_to([1, 64])
            )

            # transposed q/k for both inputs: [128, chunk(2), n]
            qT = qkpool.tile([128, 2, N], bf16, tag="qT")
            kT = qkpool.tile([128, 2, N], bf16, tag="kT")
            q1T = qkpool.tile([128, 2, N], bf16, tag="q1T")
            k1T = qkpool.tile([128, 2, N], bf16, tag="k1T")
            # v in [m, head-slices] layout, ones col per head at offset 64
            v_sb = persist.tile([128, NT, H_LOC * VW], bf16, tag="v")
            v1_sb = persist.tile([128, NT, H_LOC * VW], bf16, tag="v1")
            for vt in (v_sb, v1_sb):
                nc.vector.tensor_copy(
                    vt[:].rearrange("p t (h c) -> p t h c", h=H_LOC)[:, :, :, DHEAD:],
                    ones_f32[:, None, None, :].broadcast_to([128, NT, H_LOC, 1]),
                )

            # ---------------- filler pump machinery ----------------
            # Generators yield their approximate PE cost in ns after each
            # emitted piece.  attention() pumps a budget's worth between
            # m-chunk pairs; drain() force-finishes gens whose outputs an
            # upcoming emission reads.
            from collections import deque
            fq = deque()
            pcredit = [0.0]

            def pump(budget):
                # credit carries across m-chunk pairs so multi-ns pieces
                # average out to ~budget/mcp; the cap bounds post-idle bursts
                pcredit[0] = min(pcredit[0] + budget, 2600)
                while fq and pcredit[0] > 0:
                    try:
                        pcredit[0] -= next(fq[0])
                    except StopIteration:
                        fq.popleft()

            def drain(*gens):
                for g in gens:
                    for _ in g:
                        pass
                    if g in fq:
                        fq.remove(g)

            # ---------------- QKV projection generators ----------------
            def load_xs(srcT, half, q):
                nslc = slice(half * 1024, (half + 1) * 1024)
                xs = xstage.tile([128, KO, 1024], bf16, tag="xs")
                src_r = srcT.rearrange("(ko p) n -> p ko n", p=128)
                # split the load so early matmuls start sooner
                q.dma_start(xs[:, 0:2, :], src_r[:, 0:2, nslc])
                q.dma_start(xs[:, 2:KO, :], src_r[:, 2:KO, nslc])
                return xs

            def qk_gen(xs, qdst, kdst, half, part):
                """One q/k chunk projection as [128,512] chains on the
                transient psum ring.  part: q|q0|q1|k|k0|k1."""
                mbs = {"q": [0, 1], "q0": [0], "q1": [1],
                       "k": [2, 3], "k0": [2], "k1": [3]}[part]
                for mb in mbs:
                    dst = qdst if mb < 2 else kdst
                    ci = mb % 2
                    for nb in range(2):
                        ps = ps_out.tile([128, 512], f32, tag="ps_out")
                        for ko in range(KO):
                            nc.tensor.matmul(
                                ps[:],
                                wqkv_sb[:, ko, mb * 128:(mb + 1) * 128],
                                xs[:, ko, nb * 512:(nb + 1) * 512],
                                start=(ko == 0),
                                stop=(ko == KO - 1),
                            )
                        off = half * 1024 + nb * 512
                        nc.vector.tensor_copy(
                            dst[:, ci, off:off + 512], ps[:])
                        yield 1900

            def v_gen(xs, vdst, half):
                for nt in range(8):
                    nt_g = half * 8 + nt
                    ps = ps_out.tile([128, 512], f32, tag="ps_out")
                    for ko in range(KO):
                        nc.tensor.matmul(
                            ps[:, 0:INNER_LOC],
                            xs[:, ko, nt * 128:(nt + 1) * 128],
                            wqkv_sb[:, ko, 2 * INNER_LOC:3 * INNER_LOC],
                            start=(ko == 0),
                            stop=(ko == KO - 1),
                        )
                    nc.vector.tensor_copy(
                        vdst[:, nt_g, :]
                        .rearrange("p (h c) -> p h c", h=H_LOC)[:, :, :DHEAD],
                        ps[:, 0:INNER_LOC].rearrange("p (h c) -> p h c",
                                                     h=H_LOC),
                    )
                    yield 1400

            # ---------------- attention unit + out-projection ----------------
            ots = {}

            def attention(nb, d, hps=(0, 1), budget=1000, drain_at=None):
                """One n-block of one cross direction for one head pair."""
                nslc = slice(nb * 512, (nb + 1) * 512)
                qsrc, ksrc, vsrc = (
                    (qT, k1T, v1_sb) if d == 0 else (q1T, kT, v_sb)
                )
                if (nb, d) in ots:
                    ot = ots[(nb, d)]
                else:
                    ot = otp.tile([128, 2, 512], bf16, tag="ot")
                    ots[(nb, d)] = ot
                for hp in hps:
                    hE, hO = 2 * hp, 2 * hp + 1
                    poE = ps_o.tile([128, 512], f32, tag="po")
                    poO = ps_o.tile([128, 512], f32, tag="po")
                    for mcp in range(8):
                        if drain_at and mcp in drain_at:
                            drain(*drain_at.pop(mcp))
                        for j in range(2):
                            mc = mcp * 2 + j
                            mslc = slice(mc * 128, (mc + 1) * 128)
                            ps = ps_s.tile([128, 1024], f32, tag="ps_s")
                            nc.tensor.matmul(
                                ps[:, 0:512],
                                ksrc[0:64, hp, mslc],
                                qsrc[0:64, hp, nslc],
                                start=True, stop=True,
                            )
                            nc.tensor.matmul(
                                ps[:, 512:1024],
                                ksrc[64:128, hp, mslc],
                                qsrc[64:128, hp, nslc],
                                start=True, stop=True,
                            )
                            a = attn.tile([128, 1024], bf16, tag="a")
                            nc.scalar.activation(a[:], ps[:], AF.Exp,
                                                 scale=SCALE)
                            nc.tensor.matmul(
                                poE[0:VW, :],
                                vsrc[:, mc, hE * VW:(hE + 1) * VW],
                                a[:, 0:512],
                                start=(mc == 0), stop=(mc == NT - 1),
                            )
                            nc.tensor.matmul(
                                poO[0:VW, :],
                                vsrc[:, mc, hO * VW:(hO + 1) * VW],
                                a[:, 512:1024],
                                start=(mc == 0), stop=(mc == NT - 1),
                            )
                        pump(budget)
                    for po, prow in ((poE, slice(0, 64)),
                                     (poO, slice(64, 128))):
                        csrow = attn.tile([1, 512], f32, tag="csrow")
                        nc.vector.tensor_copy(csrow[:], po[64:65, :])
                        recip_f = attn.tile([1, 512], f32, tag="recip_f")
                        nc.vector.reciprocal_approx_fast(
                            out=recip_f[:], in_=csrow[:]
                        )
                        recip = attn.tile([1, 512], bf16, tag="recip")
                        nc.vector.tensor_copy(recip[:], recip_f[:])
                        pbt = ps_out.tile([128, 512], f32, tag="ps_out")
                        nc.tensor.matmul(pbt[0:64, :], ones_row[:], recip[:],
                                         start=True, stop=True)
                        nc.vector.tensor_copy(ot[prow, hp, :], po[0:64, :])
                        nc.vector.tensor_mul(
                            ot[prow, hp, :], ot[prow, hp, :], pbt[0:64, :]
                        )

            def proj_gen(nb, d):
                """Out-projection of one (nb, d) in per-(nt, db) pieces."""
                ot = ots[(nb, d)]
                for nt in range(4):
                    ob = outstage.tile([128, DIM], f32, tag="ob")
                    for db in range(2):
                        ps = ps_out.tile([128, 512], f32, tag="ps_out")
                        for ki in range(2):
                            nc.tensor.matmul(
                                ps[:],
                                ot[:, ki, nt * 128:(nt + 1) * 128],
                                wout_sb[:, ki, db * 512:(db + 1) * 512],
                                start=(ki == 0), stop=(ki == 1))
                        nc.vector.tensor_copy(
                            ob[:, db * 512:(db + 1) * 512], ps[:]
                        )
                        yield 600
                    obh = outstage.tile([128, DIM], bf16, tag="obh")
                    nc.vector.tensor_copy(obh[:], ob[:])
                    nc.sync.dma_start(
                        out[d, nb * 512 + nt * 128:
                            nb * 512 + (nt + 1) * 128, :],
                        obh[:],
                    )
                del ots[(nb, d)]

            def projki_gen(nb, d, ki):
                """One contraction half of (nb, d)'s projection.  ki=0
                stages f32 partials; ki=1 adds and ships out."""
                for nt in range(4):
                    if ki == 0:
                        ob = outstage.tile([128, DIM], f32, tag="ob")
                        ots[(nb, d, "ob", nt)] = ob
                    else:
                        ob = ots.pop((nb, d, "ob", nt))
                    for db in range(2):
                        ps = ps_out.tile([128, 512], f32, tag="ps_out")
                        nc.tensor.matmul(
                            ps[:],
                            ots[(nb, d)][:, ki, nt * 128:(nt + 1) * 128],
                            wout_sb[:, ki, db * 512:(db + 1) * 512],
                            start=True, stop=True)
                        if ki == 0:
                            nc.vector.tensor_copy(
                                ob[:, db * 512:(db + 1) * 512], ps[:])
                        else:
                            nc.vector.tensor_add(
                                ob[:, db * 512:(db + 1) * 512],
                                ob[:, db * 512:(db + 1) * 512], ps[:])
                        yield 300
                    if ki == 1:
                        obh = outstage.tile([128, DIM], bf16, tag="obh")
                        nc.vector.tensor_copy(obh[:], ob[:])
                        nc.sync.dma_start(
                            out[d, nb * 512 + nt * 128:
                                nb * 512 + (nt + 1) * 128, :],
                            obh[:],
                        )

            # ---------------- emission ----------------
            # Tile deps only point backward in emission order, so this is a
            # valid sequential program: every generator is force-drained
            # (or arithmetically guaranteed pumped) before the first unit
            # that reads its output.  Units run hp-major so each unit
            # introduces only a small set of new dependencies.
            xs00 = load_xs(xT, 0, nc.gpsimd)
            xs01 = load_xs(xT, 1, nc.scalar)
            xs10 = load_xs(x1T, 0, nc.sync)
            xs11 = load_xs(x1T, 1, nc.sync)
            nc.sync.dma_start(
                wout_sb[:], wout.rearrange("(ki p) d -> p ki d", p=128)
            )
            # PE warm-up during the initial DMA wait: tiny matmuls keep the
            # HAM activity monitor busy so the real prefix runs at 2.4 GHz
            # instead of the cold 1.2 GHz default.
            warm = ps_out.tile([128, 512], f32, tag="ps_out")
            for _ in range(70):
                nc.tensor.matmul(warm[0:64, 0:64], ones_row[:], ones_row[:],
                                 start=True, stop=True)
            # prefix: minimum work before the first exp can fire
            drain(qk_gen(xs00, qT, kT, 0, "k0"))
            drain(v_gen(xs00, v_sb, 0))
            drain(qk_gen(xs10, q1T, k1T, 0, "q0"))
            # U1: first unit carries its own half-1 deps in the pump queue
            g_k0h1 = qk_gen(xs01, qT, kT, 1, "k0")
            g_vh1 = v_gen(xs01, v_sb, 1)
            fq.extend([g_k0h1, g_vh1])
            attention(0, 1, hps=(0,), budget=1800,
                      drain_at={4: [g_k0h1, g_vh1]})
            g_q1h1 = qk_gen(xs11, q1T, k1T, 1, "q0")
            g_k1h0 = qk_gen(xs00, qT, kT, 0, "k1")
            g_k1h1 = qk_gen(xs01, qT, kT, 1, "k1")
            g_q1b0 = qk_gen(xs10, q1T, k1T, 0, "q1")
            fq.extend([g_q1h1, g_k1h0, g_k1h1, g_q1b0])
            attention(1, 1, hps=(0,))
            drain(g_q1h1)
            attention(2, 1, hps=(0,))
            g_q1b1 = qk_gen(xs11, q1T, k1T, 1, "q1")
            fq.append(g_q1b1)
            attention(3, 1, hps=(0,))
            drain(g_k1h0, g_k1h1, g_q1b0)
            attention(0, 1, hps=(1,))
            g_xq0 = qk_gen(xs00, qT, kT, 0, "q")
            g_k10_0 = qk_gen(xs10, q1T, k1T, 0, "k0")
            g_k11_0 = qk_gen(xs10, q1T, k1T, 0, "k1")
            g_v1h0 = v_gen(xs10, v1_sb, 0)
            fq.extend([g_xq0, g_k10_0, g_k11_0, g_v1h0])
            attention(1, 1, hps=(1,))
            g_k10h1 = qk_gen(xs11, q1T, k1T, 1, "k0")
            g_v1h1 = v_gen(xs11, v1_sb, 1)
            fq.extend([g_k10h1, g_v1h1])
            drain(g_q1b1)
            attention(2, 1, hps=(1,))
            attention(3, 1, hps=(1,))
            drain(g_xq0, g_k10_0, g_v1h0, g_v1h1)
            attention(0, 0, hps=(0,), drain_at={4: [g_k10h1]})
            g_p01 = proj_gen(0, 1)
            g_xq1 = qk_gen(xs01, qT, kT, 1, "q")
            fq.extend([g_p01, g_xq1])
            attention(1, 0, hps=(0,))
            g_p11 = proj_gen(1, 1)
            g_k11h1 = qk_gen(xs11, q1T, k1T, 1, "k1")
            fq.extend([g_p11, g_k11h1])
            # ot(2,0)/(3,0) recycle the otp ring slots of ot(0,1)/(1,1):
            # those projections must be fully emitted first
            drain(g_xq1, g_p01)
            attention(2, 0, hps=(0,))
            g_p21 = proj_gen(2, 1)
            fq.append(g_p21)
            drain(g_p11)
            attention(3, 0, hps=(0,))
            g_p31 = proj_gen(3, 1)
            fq.append(g_p31)
            drain(g_k11_0, g_k11h1)
            attention(0, 0, hps=(1,))
            g_p00 = proj_gen(0, 0)
            fq.append(g_p00)
            attention(1, 0, hps=(1,))
            g_p10 = proj_gen(1, 0)
            fq.append(g_p10)
            attention(2, 0, hps=(1,))
            g_p20 = proj_gen(2, 0)
            g_pk0 = projki_gen(3, 0, 0)
            fq.extend([g_p20, g_pk0])
            attention(3, 0, hps=(1,), budget=1400)
            drain(*list(fq))
            drain(projki_gen(3, 0, 1))
            del ots[(3, 0)]
    return nc


def _get_graph():
    if "nc" not in _CACHED:
        nc = _build_graph()
        # Bacc defers register allocation to finalize(); the pjrt exec path
        # serializes nc.m directly, so finalize here.
        nc.finalize()
        _CACHED["nc"] = nc
    return _CACHED["nc"]


def _make_in_maps(x, x1, W_qkv, W_out):
    in_maps = []
    xTb = [np.ascontiguousarray(x[b].T).astype(BF) for b in range(B)]
    x1Tb = [np.ascontiguousarray(x1[b].T).astype(BF) for b in range(B)]
    for c in range(NCORES):
        b = c // 4
        h0 = (c % 4) * H_LOC
        cols = np.concatenate(
            [W_qkv[:, j * DIM + h0 * DHEAD: j * DIM + (h0 + H_LOC) * DHEAD]
             for j in range(3)],
            axis=1,
        )
        in_maps.append({
            "xT": xTb[b],
            "x1T": x1Tb[b],
            "wqkv": np.ascontiguousarray(cols).astype(BF),
            "wout": np.ascontiguousarray(
                W_out[h0 * DHEAD:(h0 + H_LOC) * DHEAD, :]
            ).astype(BF),
        })
    return in_maps


def _run(x, x1, W_qkv, W_out, b_out, **spmd_kwargs):
    from concourse.bass_utils import run_bass_kernel_spmd

    nc = _get_graph()
    in_maps = _make_in_maps(x, x1, W_qkv, W_out)
    res = run_bass_kernel_spmd(nc, in_maps, core_ids=list(range(NCORES)),
                               **spmd_kwargs)
    parts = [r["out"].view(BF).reshape(2, N, DIM) for r in res.results]
    out = np.zeros((B, N, DIM), np.float32)
    out1 = np.zeros((B, N, DIM), np.float32)
    for b in range(B):
        grp = parts[4 * b:4 * b + 4]
        out[b] = sum(np.float32(p[0]) for p in grp) + b_out
        out1[b] = sum(np.float32(p[1]) for p in grp) + b_out
    return (out, out1), res


def kernel(x, x1, W_qkv, W_out, b_out):
    x = np.asarray(x, np.float32)
    x1 = np.asarray(x1, np.float32)
    W_qkv = np.asarray(W_qkv, np.float32)
    W_out = np.asarray(W_out, np.float32)
    b_out = np.asarray(b_out, np.float32)
    (out, out1), _ = _run(x, x1, W_qkv, W_out, b_out)
    return out, out1
